# revision 8
# baseline (speedup 1.0000x reference)
"""Cost-volume kernel for Trainium2 (Bass/Tile), SPMD over 8 NeuronCores.

volume[b, d, h, w] = mean_c left[b,c,h,w] * right[b,c,h,w-d],  0 for w < d.

Per core (one batch image b), per h row, w is split into five 64-wide blocks;
two blocks pair into one 128-partition PSUM tile:
  1. TensorE Gram matmuls: G[p, f] = sum_c L[c, w1]*Rpad[c, w2] with
     w1 = 64*wb + p, w2 = 64*wb + f - 48 (Rpad has a 48-col zero margin, so
     w < d yields exact zeros).  d = (p mod 64) - f + 48.
  2. DVE/ACT evict the [128, 112] band PSUM->SBUF (cast f32 -> bf16).
  3. An SBUF->SBUF DMA applies the skew on its DEST access pattern
     (per-partition row start = p*175 + ...), which hardware descriptor
     generation handles exactly (source-side diagonal reads do not: the byte
     component of a dirty partition step wraps every 4 partitions).
     Window cols [64:112) of the skew tile are then out[w1, 47-d].
  4. A rect DMA writes the window to DRAM out[h, w, j] (bf16).
Host: upcast bf16->f32, flip j (d = 47-j), transpose to [D, H, W].

left is pre-scaled by 1/64 on the host (exact power-of-two), folding in the
channel mean.  MM_DTYPE="f32" switches to exact fp32 end-to-end (4x slower
matmuls) as an accuracy fallback.
"""

import sys

sys.path.insert(0, "/opt/trn_rl_repo")

import numpy as np

import concourse.bass as bass
import concourse.tile as tile
from concourse import bacc, mybir
from concourse.ap import AP

B, C, H, W, D = 8, 64, 160, 320, 48
MARGIN = 48
RPAD_W = MARGIN + W          # 368
BM = 64                      # w1-block size
NBLK = W // BM               # 5
BANDW = BM + MARGIN          # 112
SKEWW = 240                  # skew tile width; col = f - p + 127

MM_DTYPE = "bf16"            # "bf16" | "f32"

_cache = {}


def _build(mm_dtype=MM_DTYPE, h_count=H, n_hchunk=2):
    in_dt = mybir.dt.bfloat16 if mm_dtype == "bf16" else mybir.dt.float32
    f32 = mybir.dt.float32

    nc = bacc.Bacc("TRN2", target_bir_lowering=False, debug=False)
    left = nc.dram_tensor("left", [C, h_count, W], in_dt, kind="ExternalInput")
    right = nc.dram_tensor("right", [C, h_count, W], in_dt, kind="ExternalInput")
    out = nc.dram_tensor("out", [h_count, W, D], in_dt, kind="ExternalOutput")

    with tile.TileContext(nc) as tc:
        with (
            tc.tile_pool(name="lt", bufs=3) as lt_pool,
            tc.tile_pool(name="rp", bufs=3) as rp_pool,
            tc.tile_pool(name="ps", bufs=6, space="PSUM") as ps_pool,
            tc.tile_pool(name="band", bufs=6) as band_pool,
            tc.tile_pool(name="skew", bufs=6) as skew_pool,
        ):
            for h0 in range(0, h_count, n_hchunk):
                hn = min(n_hchunk, h_count - h0)
                lt = lt_pool.tile([C, hn, W], in_dt)
                nc.sync.dma_start(lt[:], left[:, h0 : h0 + hn, :])
                rp = rp_pool.tile([C, hn, RPAD_W], in_dt)
                nc.gpsimd.memset(rp[:, :, 0:MARGIN].bitcast(f32), 0.0)
                nc.sync.dma_start(
                    rp[:, :, MARGIN : MARGIN + W], right[:, h0 : h0 + hn, :]
                )

                for hh in range(hn):
                    h = h0 + hh
                    for pi, wbs in enumerate([(0, 1), (2, 3), (4,)]):
                        npart = 64 * len(wbs)
                        ps = ps_pool.tile([128, BANDW], f32, tag="ps")
                        for a, wb in enumerate(wbs):
                            nc.tensor.matmul(
                                ps[64 * a : 64 * a + 64, 0:BANDW],
                                lt[:, hh, BM * wb : BM * wb + BM],
                                rp[:, hh, BM * wb : BM * wb + BANDW],
                                start=True,
                                stop=True,
                            )
                        bt = band_pool.tile([128, BANDW], in_dt, tag="band")
                        if (h * 3 + pi) % 2 == 0:
                            nc.vector.tensor_copy(
                                bt[0:npart, :], ps[0:npart, 0:BANDW]
                            )
                        else:
                            nc.scalar.copy(bt[0:npart, :], ps[0:npart, 0:BANDW])
                        # dest-skew DMA: sk[p, f - p + 127] = bt[p, f]
                        # (dirty step must be on dim0 — HW descgen handles
                        # per-partition dest row bases exactly there only)
                        sk = skew_pool.tile([128, SKEWW], in_dt, tag="skew")
                        nc.gpsimd.memset(sk[:], 0.0)
                        dst = AP(
                            sk[:].tensor, 127, [[SKEWW - 1, npart], [1, BANDW]]
                        )
                        nc.sync.dma_start(dst, bt[0:npart, :])
                        # half a: cols [128:176): col 128+j = (p, f=p+1+j) -> d = 47-j
                        nc.sync.dma_start(
                            out[h, BM * wbs[0] : BM * wbs[0] + 64, :],
                            sk[0:64, 128 : 128 + D],
                        )
                        if len(wbs) == 2:
                            # half b (p>=64): col 64+j -> d = 47-j
                            nc.sync.dma_start(
                                out[h, BM * wbs[1] : BM * wbs[1] + 64, :],
                                sk[64:128, 64 : 64 + D],
                            )

    nc.compile()
    return nc


def _get_nc():
    key = (MM_DTYPE, H)
    if key not in _cache:
        _cache[key] = _build()
    return _cache[key]


def _prep(left_feature, right_feature):
    lf = np.asarray(left_feature, dtype=np.float32) * np.float32(1.0 / C)
    rf = np.asarray(right_feature, dtype=np.float32)
    if MM_DTYPE == "bf16":
        import ml_dtypes

        lf = lf.astype(ml_dtypes.bfloat16)
        rf = rf.astype(ml_dtypes.bfloat16)
    return lf, rf


def kernel(left_feature, right_feature, disp):
    from concourse.bass_utils import run_bass_kernel_spmd

    assert int(disp) == D, f"kernel hardcoded for disp={D}, got {disp}"
    lf, rf = _prep(left_feature, right_feature)
    assert lf.shape == (B, C, H, W), lf.shape

    nc = _get_nc()
    in_maps = [{"left": lf[b], "right": rf[b]} for b in range(B)]
    res = run_bass_kernel_spmd(nc, in_maps, list(range(B)))

    vol = np.empty((B, D, H, W), dtype=np.float32)
    for b in range(B):
        o = np.asarray(res.results[b]["out"], dtype=np.float32)  # [H, W, D]
        vol[b] = o[:, :, ::-1].transpose(2, 0, 1)
    return vol


# revision 15
# speedup vs baseline: 8.1621x; 8.1621x over previous
"""Cost-volume kernel for Trainium2 (Bass/Tile), SPMD over 8 NeuronCores.

volume[b, d, h, w] = mean_c left[b,c,h,w] * right[b,c,h,w-d],  0 for w < d.

Per core (one batch image b):
  - w is split into five 64-wide blocks; blocks (2k, 2k+1) pair into one
    128-partition PSUM tile (pair k=2 is the single block 4).
  - TensorE (bf16): G[p, f] = sum_c L[c, w1]*Rpad[c, w2], w1 = 64*wb + p%64,
    w2 = 64*wb + f - 48.  Rpad has a 48-col zero margin => exact zeros for
    w < d.  d = (p%64) - f + 48.
  - DVE/ACT evict the [*, 112] band PSUM->SBUF (f32 -> bf16 cast) into a
    per-chunk wide band buffer (24 regions = 8 h-rows x 3 pairs).
  - ONE SBUF->SBUF DMA per chunk applies the skew on its DEST access pattern
    (flat dest AP [[5759,128],[240,24],[1,112]], offset 127): hardware
    descriptor generation handles per-partition dest row bases exactly.
    Region K, col 240K + (f - p + 127); windows [128:176) (rows 0:64) and
    [64:112) (rows 64:128) then hold out[w, 47-d] per half-block.
  - TWO DMAs per chunk write all windows to DRAM out2[h, hb, p, j] (bf16),
    hb = halfblock 0..5; hb=5 is garbage (pair 2 has no second block) and is
    discarded by the host.
Host: upcast bf16->f32, drop hb=5, flip j (d = 47-j), transpose to [D,H,W].

DMA count is the first-order cost on TRN2 (~0.65us sequencer issue each), so
everything is batched into per-8-row-chunk DMAs.  left is pre-scaled by 1/64
on the host (exact power of two), folding in the channel mean.
"""

import sys

sys.path.insert(0, "/opt/trn_rl_repo")

import numpy as np

import concourse.bass as bass
import concourse.tile as tile
from concourse import bacc, mybir
from concourse.ap import AP

B, C, H, W, D = 8, 64, 160, 320, 48
MARGIN = 48
RPAD_W = MARGIN + W          # 368
BM = 64                      # w1-block size
BANDW = BM + MARGIN          # 112
RW = 240                     # per-region skew width
CH = 8                       # h rows per chunk
NPAIR = 3                    # block pairs per h row
NREG = CH * NPAIR            # 24 skew regions per chunk
SKW = RW * NREG              # 5760
NSK = 3                      # rotated persistent skew buffers

MM_DTYPE = "bf16"            # "bf16" | "f32"

_cache = {}


def _build(mm_dtype=MM_DTYPE, h_count=H, reps=1):
    in_dt = mybir.dt.bfloat16 if mm_dtype == "bf16" else mybir.dt.float32
    f32 = mybir.dt.float32
    esz = 2 if mm_dtype == "bf16" else 4
    assert h_count % CH == 0
    nchunk = h_count // CH

    nc = bacc.Bacc("TRN2", target_bir_lowering=False, debug=False)
    left = nc.dram_tensor("left", [C, h_count, W], in_dt, kind="ExternalInput")
    right = nc.dram_tensor("right", [C, h_count, W], in_dt, kind="ExternalInput")
    if reps != 1:
        # unused; forces a distinct HLO per reps so the jit/NEFF caches
        # cannot alias timing builds of different rep counts
        nc.dram_tensor("rep_tag", [1, 8 * reps], mybir.dt.float32,
                       kind="ExternalInput")
    out = nc.dram_tensor("out", [h_count, 6, BM, D], in_dt, kind="ExternalOutput")

    with tile.TileContext(nc) as tc:
        sks = [
            nc.alloc_sbuf_tensor(f"skbuf{k}", [128, SKW], in_dt) for k in range(NSK)
        ]
        for sk in sks:
            nc.gpsimd.memset(sk.ap(), 0.0)
        with (
            tc.tile_pool(name="lt", bufs=3) as lt_pool,
            tc.tile_pool(name="rp", bufs=3) as rp_pool,
            tc.tile_pool(name="ps", bufs=6, space="PSUM") as ps_pool,
            tc.tile_pool(name="band", bufs=3) as band_pool,
        ):
            for ci in range(reps * nchunk):
                c = ci % nchunk
                h0 = c * CH
                lt = lt_pool.tile([C, CH, W], in_dt)
                nc.sync.dma_start(lt[:], left[:, h0 : h0 + CH, :])
                rp = rp_pool.tile([C, CH, RPAD_W], in_dt)
                nc.gpsimd.memset(rp[:, :, 0:MARGIN].bitcast(f32), 0.0)
                nc.sync.dma_start(
                    rp[:, :, MARGIN : MARGIN + W], right[:, h0 : h0 + CH, :]
                )

                bb = band_pool.tile([128, BANDW * NREG], in_dt, tag="band")
                for hh in range(CH):
                    ps = ps_pool.tile([128, BANDW * NPAIR], f32, tag="ps")
                    for k in range(NPAIR):
                        # pair 2 has no second block; duplicate block 4 into
                        # rows 64:128 (cheap) so psum/band stay fully defined
                        wbs = (2 * k, 2 * k + 1) if k < 2 else (4, 4)
                        for a, wb in enumerate(wbs):
                            nc.tensor.matmul(
                                ps[
                                    64 * a : 64 * a + 64,
                                    k * BANDW : (k + 1) * BANDW,
                                ],
                                lt[:, hh, BM * wb : BM * wb + BM],
                                rp[:, hh, BM * wb : BM * wb + BANDW],
                                start=True,
                                stop=True,
                            )
                    dstb = bb[:, NPAIR * hh * BANDW : NPAIR * (hh + 1) * BANDW]
                    if hh % 2 == 0:
                        nc.vector.tensor_copy(dstb, ps[:])
                    else:
                        nc.scalar.copy(dstb, ps[:])

                sk = sks[ci % NSK]
                # dest-skew: sk[p, 240K + f - p + 127] = bb[p, 112K + f]
                dst = AP(sk, 127, [[SKW - 1, 128], [RW, NREG], [1, BANDW]])
                nc.scalar.dma_start(
                    dst, bb[:].rearrange("p (k f) -> p k f", k=NREG)
                )
                # window-a: rows 0:64, cols 240K+[128:176) -> out[h, 2k, p, j]
                srca = AP(sk, 128, [[SKW, 64], [RW, NREG], [1, D]])
                dsta = AP(
                    out.ap().tensor,
                    c * CH * 6 * BM * D,
                    [[D, 64], [2 * BM * D, NREG], [1, D]],
                )
                nc.sync.dma_start(dsta, srca)
                # window-b: rows 64:128, cols 240K+[64:112) -> out[h, 2k+1, i, j]
                srcb = AP(sk, 64 * SKW + 64, [[SKW, 64], [RW, NREG], [1, D]])
                dstb2 = AP(
                    out.ap().tensor,
                    c * CH * 6 * BM * D + BM * D,
                    [[D, 64], [2 * BM * D, NREG], [1, D]],
                )
                nc.sync.dma_start(dstb2, srcb)

    nc.compile()
    return nc


def _get_nc():
    key = (MM_DTYPE, H)
    if key not in _cache:
        _cache[key] = _build()
    return _cache[key]


def _prep(left_feature, right_feature):
    lf = np.asarray(left_feature, dtype=np.float32) * np.float32(1.0 / C)
    rf = np.asarray(right_feature, dtype=np.float32)
    if MM_DTYPE == "bf16":
        import ml_dtypes

        lf = lf.astype(ml_dtypes.bfloat16)
        rf = rf.astype(ml_dtypes.bfloat16)
    return lf, rf


def kernel(left_feature, right_feature, disp):
    from concourse.bass_utils import run_bass_kernel_spmd

    assert int(disp) == D, f"kernel hardcoded for disp={D}, got {disp}"
    lf, rf = _prep(left_feature, right_feature)
    assert lf.shape == (B, C, H, W), lf.shape

    nc = _get_nc()
    in_maps = [{"left": lf[b], "right": rf[b]} for b in range(B)]
    res = run_bass_kernel_spmd(nc, in_maps, list(range(B)))

    vol = np.empty((B, D, H, W), dtype=np.float32)
    for b in range(B):
        o = np.asarray(res.results[b]["out"], dtype=np.float32)  # [H, 6, 64, 48]
        o = o[:, :5].reshape(H, W, D)
        vol[b] = o[:, :, ::-1].transpose(2, 0, 1)
    return vol
